# revision 2
# baseline (speedup 1.0000x reference)
"""AgentAttention block on 8 Trainium2 cores — data-parallel over batch (v2).

Per core (one batch element, x [4096, 256]):
  The q/k projections are folded into the agent matrices on the host:
  Wq_ag = Wq @ agent_bd^T and Wk_ag = Wk @ agent_bd^T, both [256, 392]
  (8 heads x 49 agents dense). So stage-1 scores s1 = x @ Wq_ag and
  stage-2-transposed scores s2T = x @ Wk_ag come straight from the
  transposed x chunks, with no q/k intermediate or PSUM evacuation.

  Per 512-row chunk: DMA-load x (f32), cast to bf16 (DVE/GPSIMD
  alternating), DMA-transpose; per 256-row pair: s1/s2 matmuls (dense
  392-col accumulation over the two 128-dim planes), v matmuls, exp on
  ACT (no max-subtraction needed: |scores*scale| <~ 2), stage-1 softmax
  row-sums on DVE (bf16 out), reciprocal, normalize-mul (DVE/GPSIMD
  alternating pairs), v evac (DVE/ACT alternating chunks), stage-2
  pooling + denominator via an accumulated matmul against v augmented
  with a ones column. e1n is transposed per chunk via bf16 DMA-transpose.

  Tail: M = x_a / c2 per agent, MW = M @ Wproj (tiny), then the final
  mix out = E1nT.T @ MW with ACT copy-evac to bf16 and bf16 DRAM store
  (host widens to f32). Stage-2 score bias is mathematically irrelevant
  (uniform shift inside the softmax over n); stage-1/v/proj biases are
  all zero in this model instance, and nonzero variants enable extra
  build paths (w1 multiplicative weights, bias slot in MW).
"""
import numpy as np
import ml_dtypes
import concourse.bass as bass
import concourse.tile as tile
from concourse import bacc, mybir
from concourse.bass_utils import run_bass_kernel_spmd
from contextlib import ExitStack

B, N, DIM = 8, 4096, 256
H, HD, A = 8, 32, 49
HA = H * A
SCALE = float(HD) ** -0.5
NCORES = 8
CHUNKS, CW, SUBS = 8, 512, 4
BF = mybir.dt.bfloat16
F32 = mybir.dt.float32
AF = mybir.ActivationFunctionType
ALU = mybir.AluOpType


def build_nc(cfg=None, dbg=False):
    cfg = {**dict(warmup=0, warmup2=0, split0=2, tde1n="sync", store="gps",
                  has_t1=False, has_bp=False, r1f32=False, mul="alt",
                  cast="pda", vat="dve", fo="dve", xb=4, xfb=8,
                  xldq="mix", tep="chunk"),
           **(cfg or {})}
    nc = bacc.Bacc("TRN2", target_bir_lowering=False, debug=False)
    x = nc.dram_tensor("x", [N, DIM], F32, kind="ExternalInput").ap()
    wqag = nc.dram_tensor("wqag", [128, 2, HA], BF, kind="ExternalInput").ap()
    wkag = nc.dram_tensor("wkag", [128, 2, HA], BF, kind="ExternalInput").ap()
    wv = nc.dram_tensor("wv", [128, 2, 256], BF, kind="ExternalInput").ap()
    wp = nc.dram_tensor("wp", [64, 4, 256], BF, kind="ExternalInput").ap()
    eye = nc.dram_tensor("eye", [128, 128], BF, kind="ExternalInput").ap()
    out = nc.dram_tensor("out", [N, DIM], BF, kind="ExternalOutput").ap()
    if dbg:
        d_e1raw = nc.dram_tensor("d_e1raw", [128, 2, 8, A], BF,
                                 kind="ExternalOutput").ap()
        d_e1n = nc.dram_tensor("d_e1n", [128, 4, 512], BF,
                               kind="ExternalOutput").ap()
        d_e2 = nc.dram_tensor("d_e2", [128, 4, 512], BF,
                              kind="ExternalOutput").ap()
        d_va = nc.dram_tensor("d_va", [128, 2, 4, 65], BF,
                              kind="ExternalOutput").ap()
        d_e1t = nc.dram_tensor("d_e1t", [128, 4, 4, 128], BF,
                               kind="ExternalOutput").ap()
        d_nm = nc.dram_tensor("d_nm", [128, 4, 65], F32,
                              kind="ExternalOutput").ap()
        d_mb = nc.dram_tensor("d_mb", [128, 4, 64], BF,
                              kind="ExternalOutput").ap()
        d_mw = nc.dram_tensor("d_mw", [128, 4, 256], BF,
                              kind="ExternalOutput").ap()
    if cfg["has_t1"]:
        w1r = nc.dram_tensor("w1r", [128, HA], BF, kind="ExternalInput").ap()
    if cfg["has_bp"]:
        bpr = nc.dram_tensor("bpr", [128, 256], F32, kind="ExternalInput").ap()

    with tile.TileContext(nc) as tc, ExitStack() as ctx:
        const = ctx.enter_context(tc.tile_pool(name="const", bufs=1))
        pers = ctx.enter_context(tc.tile_pool(name="pers", bufs=1))
        xfp = ctx.enter_context(tc.tile_pool(name="xfp", bufs=cfg["xfb"]))
        xsp = ctx.enter_context(tc.tile_pool(name="xsp", bufs=cfg["xb"]))
        xtp = ctx.enter_context(tc.tile_pool(name="xtp", bufs=cfg["xb"]))
        vap = ctx.enter_context(tc.tile_pool(name="vap", bufs=3))
        ep = ctx.enter_context(tc.tile_pool(name="ep", bufs=3))
        rp = ctx.enter_context(tc.tile_pool(name="rp", bufs=3))
        fop = ctx.enter_context(tc.tile_pool(name="fop", bufs=3))
        ctxA = ExitStack()
        s1p = ctxA.enter_context(tc.tile_pool(name="s1p", bufs=2, space="PSUM"))
        s2p = ctxA.enter_context(tc.tile_pool(name="s2p", bufs=1, space="PSUM"))
        vp = ctxA.enter_context(tc.tile_pool(name="vp", bufs=1, space="PSUM"))
        nmp = ctxA.enter_context(tc.tile_pool(name="nmp", bufs=1, space="PSUM"))

        # e1t_all[p, cnk, t, g, c] = E1n[cnk*512+128t+c, 128g+p]
        e1t_all = pers.tile([128, CHUNKS, 4, 4, 128], BF, tag="e1t")
        pnm = nmp.tile([128, 4, 65], F32, tag="nm")

        # ---- rolling x prefetch: load + cast + transpose ----
        xT_list = []

        def load_x(cnk):
            n0 = cnk * CW
            xf = xfp.tile([128, SUBS, DIM], F32, tag="xf")
            xbf = xsp.tile([128, SUBS, DIM], BF, tag="xbf")
            xTc = xtp.tile([128, SUBS, 2, 128], BF, tag="xT")
            # keep the (serializing) sync DMA queue for xbar transposes only:
            # x loads ride the scalar/gpsimd queues
            ld_eng = {"sync": nc.sync, "gps": nc.gpsimd,
                      "mix": nc.scalar if cnk % 2 == 0 else nc.gpsimd
                      }[cfg["xldq"]]
            if cnk < cfg["split0"]:
                # fine-grained start: per-sub-tile loads spread across three
                # DMA queues in parallel (a single queue sustains only
                # ~65 GB/s) + per-sub casts. The transpose itself stays
                # whole-chunk: back-to-back xbar transposes corrupt on
                # hardware (single shared xbar).
                engs = [nc.scalar, nc.sync, nc.gpsimd]
                for t in range(SUBS):
                    r0 = n0 + 128 * t
                    engs[(t + cnk) % 3].dma_start(xf[:, t, :], x[r0:r0 + 128, :])
                    if cfg["cast"] != "none":
                        nc.vector.tensor_copy(xbf[:, t, :], xf[:, t, :])
            else:
                ld_eng.dma_start(
                    xf[:], x[n0:n0 + CW, :].rearrange("(t p) c -> p t c", p=128))
                if cfg["cast"] == "none":
                    pass
                elif cnk % 2 == 0:
                    nc.vector.tensor_copy(xbf[:], xf[:])
                elif cfg["cast"] == "gps":
                    nc.gpsimd.tensor_copy(xbf[:], xf[:])
                else:
                    nc.scalar.copy(xbf[:], xf[:])
            # xTc[p, t, kb, c] = x[n0+128t+c, 128kb+p]
            if cfg["cast"] == "none":
                # no cast: the xbar transpose reads the high u16 half of each
                # f32 directly (truncating round to bf16)
                tin = xf[:].bitcast(BF) \
                    .rearrange("p t (c two) -> p t c two", two=2)[:, :, :, 1] \
                    .rearrange("p t c -> p (t c)")
            else:
                tin = xbf[:].rearrange("p t c -> p (t c)")
            nc.sync.dma_start(xTc[:].rearrange("p t k c -> p (t k) c"),
                              tin, transpose=True)
            xT_list.append(xTc)

        # weights first: tiny DMAs that must not crawl behind x loads
        wqag_sb = const.tile([128, 2, HA], BF, tag="wqag")
        nc.scalar.dma_start(wqag_sb[:], wqag[:])
        wkag_sb = const.tile([128, 2, HA], BF, tag="wkag")
        nc.scalar.dma_start(wkag_sb[:], wkag[:])
        wv_sb = const.tile([128, 2, 256], BF, tag="wv")
        nc.scalar.dma_start(wv_sb[:], wv[:])
        wp_sb = const.tile([64, 4, 256], BF, tag="wp")
        nc.scalar.dma_start(wp_sb[:], wp[:])
        eye_sb = const.tile([128, 128], BF, tag="eye")
        nc.scalar.dma_start(eye_sb[:], eye[:])

        for cnk in range(2):
            load_x(cnk)
        if cfg["has_t1"]:
            w1_sb = const.tile([128, HA], BF, tag="w1")
            nc.scalar.dma_start(w1_sb[:], w1r[:])
        if cfg["has_bp"]:
            bp_sb = const.tile([128, 256], F32, tag="bp")
            nc.scalar.dma_start(bp_sb[:], bpr[:])

        wmt = None
        if cfg["warmup"] or cfg["warmup2"]:
            wmt = const.tile([128, 512], BF, tag="wmt")
            nc.gpsimd.memset(wmt[:], 0.0)

        def warm(n):
            # short dummy matmuls to hold the PE p-state; rides the s1 psum ring
            wpt = s1p.tile([128, 2, 512], F32, tag="s1")
            for i in range(n):
                nc.tensor.matmul(wpt[:, i % 2, 0:128], wmt[:, 0:128],
                                 wmt[:, 0:128], start=(i < 2), stop=(i >= n - 2),
                                 skip_group_check=True)

        if cfg["warmup"]:
            warm(cfg["warmup"])

        for cnk in range(2, CHUNKS):
            load_x(cnk)

        # ---- Loop A: scores, exps, v, stage-2 pooling ----
        # nm matmuls for pair i are deferred until after pair i+1's score
        # matmuls are issued: the PE queue never waits on the exp of the
        # pair it just produced.
        pending_nm = []

        def flush_nm():
            for e2t, vat_, cnk_, pr_ in pending_nm:
                for st in (0, 1):
                    t = 2 * pr_ + st
                    i = cnk_ * SUBS + t
                    for g in range(4):
                        # exactly one start=True for the nm bank
                        nc.tensor.matmul(
                            pnm[:, g, :],
                            e2t[:, t, 128 * g:128 * (g + 1)],
                            vat_[:, st, g, :],
                            start=(i == 0 and g == 0), stop=(i == 31),
                            skip_group_check=True)
            pending_nm.clear()

        for cnk in range(CHUNKS):
            xTc = xT_list[cnk]
            e2_c = ep.tile([128, SUBS, 512], BF, tag="e2")
            if cfg["tep"] == "chunk":
                e1n_c = ep.tile([128, SUBS, 512], BF, tag="e1nc")
            else:
                e1n_c = None
            for pr in range(2):  # pairs of 128-row sub-tiles
                st0 = 2 * pr
                if e1n_c is not None:
                    e1n_p = e1n_c[:, st0:st0 + 2, :]
                else:
                    e1n_t = ep.tile([128, 2, 512], BF, tag="e1n")
                    e1n_p = e1n_t[:]
                # v for the pair, one psum bank
                pv = vp.tile([128, 2, 256], F32, tag="pv")
                for st in (0, 1):
                    t = st0 + st
                    nc.tensor.matmul(pv[:, st, :], xTc[:, t, 0, :], wv_sb[:, 0, :],
                                     start=(st == 0), stop=False,
                                     skip_group_check=True)
                    nc.tensor.matmul(pv[:, st, :], xTc[:, t, 1, :], wv_sb[:, 1, :],
                                     start=False, stop=(st == 1),
                                     skip_group_check=True)
                vat = vap.tile([128, 2, 4, 65], BF, tag="va")
                if cfg["vat"] == "alt" and cnk % 2 == 1:
                    nc.scalar.activation(
                        vat[:, :, :, 0:64],
                        pv[:].rearrange("p s (g d) -> p s g d", g=4), AF.Copy)
                else:
                    nc.vector.tensor_copy(
                        vat[:, :, :, 0:64],
                        pv[:].rearrange("p s (g d) -> p s g d", g=4))
                nc.gpsimd.memset(vat[:, :, :, 64:65], 1.0)

                ps1 = s1p.tile([128, 2, 512], F32, tag="s1")
                ps2 = s2p.tile([128, 2, 512], F32, tag="s2")
                for st in (0, 1):
                    t = st0 + st
                    for kb in range(2):
                        nc.tensor.matmul(ps1[:, st, 0:HA], xTc[:, t, kb, :],
                                         wqag_sb[:, kb, :],
                                         start=(kb == 0), stop=(kb == 1),
                                         skip_group_check=True)
                        nc.tensor.matmul(ps2[:, st, 0:HA], xTc[:, t, kb, :],
                                         wkag_sb[:, kb, :],
                                         start=(kb == 0), stop=(kb == 1),
                                         skip_group_check=True)
                flush_nm()

                e2v = e2_c[:, st0:st0 + 2, :].rearrange("p s (h j) -> p s h j", h=8)
                nc.scalar.activation(
                    e2v[:, :, :, 0:A], ps_view(ps2), AF.Exp, scale=SCALE)
                e1raw = rp.tile([128, 2, 8, A], BF, tag="e1raw")
                nc.scalar.activation(e1raw[:], ps_view(ps1), AF.Exp, scale=SCALE)
                if cfg["has_t1"]:
                    nc.vector.tensor_mul(
                        e1raw[:], e1raw[:],
                        w1_sb[:].rearrange("p (h j) -> p 1 h j", h=8)
                        .to_broadcast((128, 2, 8, A)))

                r1 = rp.tile([128, 2, 8], F32 if cfg["r1f32"] else BF, tag="r1")
                with nc.allow_low_precision(reason="softmax denom fits bf16"):
                    nc.vector.tensor_reduce(r1[:], e1raw[:],
                                            axis=mybir.AxisListType.X, op=ALU.add)
                r1i = rp.tile([128, 2, 8], F32 if cfg["r1f32"] else BF, tag="r1i")
                with nc.allow_low_precision(reason="softmax denom fits bf16"):
                    nc.vector.reciprocal(r1i[:], r1[:])
                e1v = e1n_p.rearrange("p s (h j) -> p s h j", h=8)
                mul_eng = nc.vector if (pr == 0 or cfg["mul"] == "dve") \
                    else nc.gpsimd
                mul_eng.tensor_mul(
                    e1v[:, :, :, 0:A], e1raw[:],
                    r1i[:].rearrange("p s (h o) -> p s h o", o=1)
                        .to_broadcast((128, 2, 8, A)))
                nc.gpsimd.memset(e1v[:, :, :, A:64], 0.0)
                if cfg["has_bp"]:
                    nc.gpsimd.memset(e1n_p[:, :, 63:64], 1.0)
                if dbg and cnk == 0 and pr == 0:
                    nc.sync.dma_start(d_e1raw[:], e1raw[:])
                    nc.sync.dma_start(d_va[:], vat[:])
                if dbg and cnk == 0:
                    nc.sync.dma_start(d_e1n[:, st0:st0 + 2], e1n_p)

                if cfg["tep"] == "pair":
                    nc.sync.dma_start(
                        e1t_all[:, cnk, st0:st0 + 2]
                        .rearrange("p t g c -> p (t g) c"),
                        e1n_p.rearrange("p s f -> p (s f)"), transpose=True)

                pending_nm.append((e2_c, vat, cnk, pr))

            if cfg["tep"] == "chunk":
                nc.sync.dma_start(
                    e1t_all[:, cnk].rearrange("p t g c -> p (t g) c"),
                    e1n_c[:].rearrange("p t f -> p (t f)"), transpose=True)
            if dbg and cnk == 0:
                nc.sync.dma_start(d_e2[:], e2_c[:])

        flush_nm()
        if dbg:
            nc.sync.dma_start(d_e1t[:], e1t_all[:, 0])
            d_nm_sb = pers.tile([128, 4, 65], F32, tag="dnm")
            nc.vector.tensor_copy(d_nm_sb[:], pnm[:])
            nc.sync.dma_start(d_nm[:], d_nm_sb[:])
        if cfg["warmup2"]:
            warm(cfg["warmup2"])
        # ---- M = x_a / c2 (per agent), block layout for the final mix ----
        mblk = pers.tile([128, 4, 64], BF, tag="mblk")
        nc.vector.memset(mblk[:], 0.0)
        for g in range(4):
            c2i = rp.tile([128, 1], F32, tag="c2i")
            nc.vector.reciprocal(c2i[:], pnm[:, g, 64:65])
            nc.vector.tensor_scalar_mul(mblk[0:A, g, 0:32], pnm[0:A, g, 0:32],
                                        c2i[0:A, 0:1])
            nc.vector.tensor_scalar_mul(mblk[64:64 + A, g, 32:64],
                                        pnm[64:64 + A, g, 32:64],
                                        c2i[64:64 + A, 0:1])
        ctxA.close()
        rotB = ctx.enter_context(tc.tile_pool(name="rotB", bufs=6, space="PSUM"))
        # ---- MW = M @ Wproj (tiny): transpose M blocks, then 4 matmuls ----
        mbt = pers.tile([64, 4, 128], BF, tag="mbt")
        for g in range(4):
            tp = rotB.tile([64, 128], BF, tag="rotB")
            nc.tensor.transpose(tp[:], mblk[:, g, :], eye_sb[:])
            nc.vector.tensor_copy(mbt[:, g, :], tp[:])
        mw = pers.tile([128, 4, 256], BF, tag="mw")
        for g in range(4):
            pw = rotB.tile([128, 256], F32, tag="rotB")
            nc.tensor.matmul(pw[:], mbt[:, g, :], wp_sb[:, g, :],
                             start=True, stop=True, skip_group_check=True)
            nc.scalar.activation(mw[:, g, :], pw[:], AF.Copy)
        if cfg["has_bp"]:
            # slot 63 of head 0 carries the effective output bias
            nc.vector.tensor_copy(mw[63:64, 0, :], bp_sb[63:64, :])
        if dbg:
            nc.sync.dma_start(d_mb[:], mblk[:])
            nc.sync.dma_start(d_mw[:], mw[:])
        # ---- Loop B: out = E1nT.T @ MW ----
        for cnk in range(CHUNKS):
            n0 = cnk * CW
            fo_c = fop.tile([128, SUBS, 256], BF, tag="fo")
            for t in range(SUBS):
                pf = rotB.tile([128, 256], F32, tag="rotB")
                for g in range(4):
                    nc.tensor.matmul(pf[:], e1t_all[:, cnk, t, g, :],
                                     mw[:, g, :], start=(g == 0), stop=(g == 3),
                                     skip_group_check=True)
                if cfg["fo"] == "act":
                    nc.scalar.activation(fo_c[:, t, :], pf[:], AF.Copy)
                else:
                    nc.vector.tensor_copy(fo_c[:, t, :], pf[:])
            st_eng = nc.sync if cfg["store"] == "sync" else nc.gpsimd
            st_eng.dma_start(
                out[n0:n0 + CW, :].rearrange("(t p) c -> p t c", p=128), fo_c[:])

    nc.compile()
    return nc


def ps_view(ps):
    return ps[:, :, 0:HA].rearrange("p s (h j) -> p s h j", h=8)


_NC = {}
CFG = {}  # module-level overrides for experiments (set before first kernel())


def _get_nc(key=(False, False), cfg=None):
    if key not in _NC:
        has_t1, has_bp = key
        _NC[key] = build_nc({**CFG, **(cfg or {}),
                             "has_t1": has_t1, "has_bp": has_bp})
    return _NC[key]


def _prep_consts(Wq, bq, Wkv, bkv, agent_p, Wproj, bproj):
    bfd = ml_dtypes.bfloat16
    f32 = np.float32
    f64 = np.float64

    ag = agent_p.reshape(A, DIM).astype(f64)
    wqag_h = np.zeros((DIM, HA), f64)
    wkag_h = np.zeros((DIM, HA), f64)
    wk = Wkv[:, 0:DIM].astype(f64)
    wq64 = Wq.astype(f64)
    for h in range(H):
        d = slice(HD * h, HD * h + HD)
        wqag_h[:, A * h:A * h + A] = wq64[:, d] @ ag[:, d].T
        wkag_h[:, A * h:A * h + A] = wk[:, d] @ ag[:, d].T

    def pack(w):  # [256, HA] -> [128, kb, HA]
        return np.ascontiguousarray(w.reshape(2, 128, HA).transpose(1, 0, 2))

    wv_h = np.ascontiguousarray(
        Wkv[:, DIM:2 * DIM].reshape(2, 128, 256).transpose(1, 0, 2)).astype(bfd)
    wp_h = np.ascontiguousarray(
        Wproj.reshape(4, 64, 256).transpose(1, 0, 2)).astype(bfd)
    eye_h = np.eye(128).astype(bfd)

    consts = {"wqag": pack(wqag_h).astype(bfd), "wkag": pack(wkag_h).astype(bfd),
              "wv": wv_h, "wp": wp_h, "eye": eye_h}

    # stage-1 score bias -> multiplicative softmax weights (zero in this model)
    t1 = np.zeros(HA, f64)
    for h in range(H):
        d = slice(HD * h, HD * h + HD)
        t1[A * h:A * h + A] = ag[:, d] @ bq[d].astype(f64)
    has_t1 = bool(np.abs(t1).max() > 0)
    if has_t1:
        w1 = np.exp(SCALE * t1)
        consts["w1r"] = np.ascontiguousarray(
            np.broadcast_to(w1, (128, HA))).astype(bfd)

    # v-bias + proj bias fold (zero in this model)
    bp_eff = bproj.astype(f64) + bkv[DIM:2 * DIM].astype(f64) @ Wproj.astype(f64)
    has_bp = bool(np.abs(bp_eff).max() > 0)
    if has_bp:
        consts["bpr"] = np.ascontiguousarray(
            np.broadcast_to(bp_eff, (128, 256))).astype(f32)
    return consts, (has_t1, has_bp)


def kernel(**inputs):
    x = np.asarray(inputs["x"], np.float32)
    consts, key = _prep_consts(
        np.asarray(inputs["Wq"], np.float32),
        np.asarray(inputs["bq"], np.float32),
        np.asarray(inputs["Wkv"], np.float32),
        np.asarray(inputs["bkv"], np.float32),
        np.asarray(inputs["agent_p"], np.float32),
        np.asarray(inputs["Wproj"], np.float32),
        np.asarray(inputs["bproj"], np.float32),
    )
    in_maps = [{**consts, "x": np.ascontiguousarray(x[b])} for b in range(B)]
    nc = _get_nc(key)
    res = run_bass_kernel_spmd(nc, in_maps, list(range(NCORES)))
    return np.stack([np.asarray(res.results[b]["out"], np.float32)
                     for b in range(B)], axis=0)
